# revision 42
# baseline (speedup 1.0000x reference)
"""MoE downsample kernel for 8 TRN2 NeuronCores — top-2 sparse.

The reference computes all 4 experts densely, but only the top-2 gated
experts per sample contribute to the output. Gating depends only on the
input mean-pool, so it is computed on host BEFORE compiling the device
program; the program then contains matmuls only for the selected
(sample, expert) pairs (~half the dense FLOPs for typical gatings).

Sharding: every selected (sample, expert) conv is band-sharded across
all 8 cores — core c computes output rows [16c, 16c+16). All cores
therefore execute an IDENTICAL instruction stream (SPMD-safe); only the
staged input rows differ per core. Within a core, samples are processed
in rounds streamed through SBUF: samples are exactly balanced
(subset-sum over gating costs) onto the two PE row-halves (partitions
0-63 / 64-127), and each sample's two experts are split across the two
PE col-halves so four 64x64 tile_position matmuls run concurrently
(full 128x128 array). Each strided dilated
conv is decomposed into k*k "tap" matmuls accumulated in PSUM over
512-pixel chunks; BN + conv-bias + GELU fuse into the ScalarE PSUM
eviction. Top-2 weighting and concat run on host.
"""

import numpy as np
import ml_dtypes

KS = [3, 5, 7, 9]
DS = [1, 2, 3, 4]
HALO = [d * (k - 1) // 2 for k, d in zip(KS, DS)]  # [1, 4, 9, 16]
BN_EPS = 1e-5
B, CIN, H, W = 16, 64, 256, 256
CE = 64
PAD = 16           # left/top pad (max halo); right/bottom needs 15
HP = WP = PAD + 256 + 15   # 287
HO = WO = 128
NCORES = 8
BAND = 16          # output rows per core per (sample, expert)
CHUNK_ROWS = 4     # output rows per 512-px PSUM chunk
NCHUNK = BAND // CHUNK_ROWS   # 4 chunks per (sample, expert) band
RMAX = 31 + 2 * max(HALO)     # 63 input rows per staged piece (max)
NTAPS = sum(k * k for k in KS)  # 164

# tap slot base per expert in the packed weight tensor
_SLOT_BASE = np.cumsum([0] + [k * k for k in KS]).tolist()

_CACHE = {}


def _tap_offsets(e):
    """Yield (slot, row_off, col_off) in padded coords for expert e."""
    k, d = KS[e], DS[e]
    pad = d * (k - 1) // 2
    for u in range(k):
        for v in range(k):
            slot = _SLOT_BASE[e] + u * k + v
            yield slot, d * u - pad + PAD, d * v - pad + PAD


def _make_schedule(idx):
    """Build the shared (all-core) round/queue schedule from gating."""
    idx = [(int(a), int(b)) for a, b in idx]
    costs = [KS[a] ** 2 + KS[b] ** 2 for a, b in idx]
    # exact-balance partition of samples onto the two PE row halves
    # (subset-sum DP over the 16 sample costs), fallback to LPT
    total = sum(costs)
    target = total // 2
    reach = {0: []}
    for s in range(B):
        upd = {}
        for v, mem in reach.items():
            nv = v + costs[s]
            if nv <= target and nv not in reach and nv not in upd:
                upd[nv] = mem + [s]
        reach.update(upd)
    bestv = max(reach)
    h0 = set(reach[bestv])
    halves = [sorted(h0), [s for s in range(B) if s not in h0]]
    loads = [bestv, total - bestv]
    # h0 round 0 light-ish (fast start, but window long enough that the
    # round-1 piece DMA lands in time), then heavy rounds early; h1
    # heavy-first
    halves[0].sort(key=lambda s: costs[s])
    halves[0] = halves[0][:1] + sorted(halves[0][1:], key=lambda s: -costs[s])
    halves[1].sort(key=lambda s: -costs[s])
    rounds = [[], []]                    # per half: (sample, halo, R, roff)
    rtot = [0, 0]
    for h in (0, 1):
        for s in halves[h]:
            halo = max(HALO[e] for e in idx[s])
            r_rows = 31 + 2 * halo
            rounds[h].append((s, halo, r_rows, rtot[h]))
            rtot[h] += r_rows
    # chunk ids (output slots) + per-queue per-round chunk lists
    chunk_map = []                       # cid -> (sample, expert, j, ch)
    queue_chunks = {}                    # (h, ch) -> [per-round [(e, j, cid)]]
    for h in (0, 1):
        for ch in (0, 1):
            queue_chunks[(h, ch)] = []
    for h in (0, 1):
        for (s, halo, r_rows, roff) in rounds[h]:
            ea, eb = idx[s]
            for ch in (0, 1):
                # first chunk of each queue touches only the first row slab
                lst = ([(ea, 0), (ea, 2), (eb, 1), (eb, 3)] if ch == 0
                       else [(ea, 1), (ea, 3), (eb, 0), (eb, 2)])
                entry = []
                for (e, j) in lst:
                    cid = len(chunk_map)
                    chunk_map.append((s, e, j, ch))
                    entry.append((e, j, cid))
                queue_chunks[(h, ch)].append(entry)
    return dict(idx=idx, halves=halves, loads=loads, rounds=rounds,
                rtot=rtot, chunk_map=chunk_map, queue_chunks=queue_chunks)


def _build_program(sched):
    import concourse.bass as bass  # noqa: F401
    import concourse.mybir as mybir
    import concourse.tile as tile
    from concourse import bacc
    from contextlib import ExitStack

    dt = mybir.dt
    nc = bacc.Bacc("TRN2", target_bir_lowering=False, debug=False,
                   num_devices=NCORES)
    xp = [nc.dram_tensor(f"xp{h}", [CIN, max(sched["rtot"][h], 1), WP],
                         dt.bfloat16, kind="ExternalInput") for h in (0, 1)]
    wt = nc.dram_tensor("wt", [CIN, NTAPS, CE], dt.bfloat16,
                        kind="ExternalInput")
    bnp = nc.dram_tensor("bnp", [CE, 4, 2], dt.float32, kind="ExternalInput")
    ncid = len(sched["chunk_map"])
    out = nc.dram_tensor("out", [ncid, CE, CHUNK_ROWS, WO], dt.bfloat16,
                         kind="ExternalOutput")

    with tile.TileContext(nc) as tc:
        with ExitStack() as ctx:
            consts = ctx.enter_context(tc.tile_pool(name="consts", bufs=1))
            px0 = ctx.enter_context(tc.tile_pool(name="px0", bufs=2))
            px1 = ctx.enter_context(tc.tile_pool(name="px1", bufs=2))
            piece_pools = [px0, px1]
            stage_pool = ctx.enter_context(tc.tile_pool(name="st", bufs=8))

            wtile = consts.tile([128, NTAPS, CE], dt.bfloat16)
            bntile = consts.tile([128, 4, 2], dt.float32)

            psum_pool = ctx.enter_context(
                tc.tile_pool(name="ps", bufs=8, space="PSUM"))

            piece_shared = [{}, {}]   # h -> round -> sbuf tile

            def stage_piece(h, r, lo, hi, alloc=False, eng=None):
                """DMA rows [lo,hi) of the round-r piece for half h."""
                s, halo, r_rows, roff = sched["rounds"][h][r]
                p0 = h * 64
                hi = min(hi, r_rows)
                if alloc:
                    pt = piece_pools[h].tile([128, RMAX, WP], dt.bfloat16)
                    piece_shared[h][r] = pt
                else:
                    pt = piece_shared[h][r]
                if lo >= hi:
                    return
                (eng or nc.gpsimd).dma_start(
                    out=pt[p0:p0 + 64, lo:hi, :],
                    in_=xp[h][:, roff + lo:roff + hi, :])

            def stage_weights(h, e, t0=0, t1=None, eng=None):
                p0 = h * 64
                sb = _SLOT_BASE[e]
                ke = KS[e] * KS[e]
                t1 = ke if t1 is None else min(t1, ke)
                if t0 >= t1:
                    return
                (eng or nc.gpsimd).dma_start(
                    out=wtile[p0:p0 + 64, sb + t0:sb + t1, :],
                    in_=wt[:, sb + t0:sb + t1, :])

            # ---- prologue: order DMAs so the first matmuls start early ----
            first_use = [[], []]       # per half: experts by first use
            for h in (0, 1):
                for (s, _h_, _r_, _o_) in sched["rounds"][h]:
                    for e in sched["idx"][s]:
                        if e not in first_use[h]:
                            first_use[h].append(e)
            hw = nc.gpsimd
            # critical path: h0 round-0 first-chunk deps, then h1's
            for h in (0, 1):
                if not sched["rounds"][h]:
                    continue
                halo0 = sched["rounds"][h][0][1]
                stage_weights(h, first_use[h][0], 0, 16, eng=hw)
                stage_piece(h, 0, 0, 7 + 2 * halo0, alloc=True, eng=hw)
            # bn params gate every eviction (needed ~6us after first MM)
            for h in (0, 1):
                hw.dma_start(out=bntile[h * 64:h * 64 + 64, :, :],
                             in_=bnp.ap())
            # warm the PE clock (HAM) with dummy matmuls while DMA streams;
            # the psum slot recycles via the pool ring, it is never read
            if first_use[0]:
                wsb = _SLOT_BASE[first_use[0][0]]
                ps = psum_pool.tile([128, 512], dt.float32)
                for _ in range(14):
                    nc.tensor.matmul(ps[0:64, 0:64],
                                     wtile[0:64, wsb, :],
                                     wtile[0:64, wsb, :],
                                     start=True, stop=True,
                                     tile_position=(0, 0))
            for h in (0, 1):
                if not sched["rounds"][h]:
                    continue
                halo0 = sched["rounds"][h][0][1]
                stage_weights(h, first_use[h][0], 16, None, eng=hw)
                stage_piece(h, 0, 7 + 2 * halo0, 15 + 2 * halo0, eng=hw)
            for h in (0, 1):           # second expert + rest of round 0
                if len(first_use[h]) > 1:
                    stage_weights(h, first_use[h][1], eng=hw)
                if sched["rounds"][h]:
                    halo0 = sched["rounds"][h][0][1]
                    stage_piece(h, 0, 15 + 2 * halo0, RMAX, eng=hw)
            # experts first needed in round >= 1 are staged from the pump,
            # one round ahead — keeps the prologue DMA backlog small
            first_round = [{}, {}]
            for h in (0, 1):
                for r, (s, _h_, _r_, _o_) in enumerate(sched["rounds"][h]):
                    for e in sched["idx"][s]:
                        first_round[h].setdefault(e, r)

            def queue_events(h, ch):
                p0 = h * 64            # rhs/lhsT partitions (PE rows)
                q0 = ch * 64           # psum/out partitions (PE cols)
                for r, (s, halo, r_rows, roff) in enumerate(
                        sched["rounds"][h]):
                    if ch == 0 and r + 1 < len(sched["rounds"][h]):
                        # prefetch next round's piece + its new experts'
                        # weights, critical-first: first taps of the first
                        # expert and the first row slab ahead of the bulk
                        h1rows = 15 + 2 * sched["rounds"][h][r + 1][1]
                        wes = [e for e, fr in first_round[h].items()
                               if fr == r + 1]
                        a_next = sched["idx"][sched["rounds"][h][r + 1][0]][0]

                        def stage(h=h, r=r, h1rows=h1rows, wes=wes,
                                  a_next=a_next):
                            # HWDGE (ACT ring) bypasses the SWDGE FIFO, so
                            # these never queue behind prologue bulk
                            if a_next in wes:
                                stage_weights(h, a_next, 0, 16, eng=nc.scalar)
                            stage_piece(h, r + 1, 0, h1rows, alloc=True,
                                        eng=nc.scalar)
                            if a_next in wes:
                                stage_weights(h, a_next, 16, None,
                                              eng=nc.scalar)
                            for e in wes:
                                if e != a_next:
                                    stage_weights(h, e, eng=nc.scalar)
                            stage_piece(h, r + 1, h1rows, RMAX,
                                        eng=nc.scalar)
                        yield ("dma", stage)
                    else:
                        yield ("noop", lambda: None)  # keep queues in lockstep
                    def mk_mm(j, ps, first, last, slot, ro, co,
                              h=h, r=r, halo=halo, p0=p0, q0=q0):
                        rl = 8 * j + ro - PAD + halo

                        def mm():
                            pt = piece_shared[h][r]
                            rhs = pt[p0:p0 + 64,
                                     rl:rl + 2 * CHUNK_ROWS - 1:2,
                                     co:co + 2 * WO - 1:2]
                            lhsT = wtile[p0:p0 + 64, slot, :]
                            nc.tensor.matmul(ps[q0:q0 + 64, :], lhsT, rhs,
                                             start=first, stop=last,
                                             tile_position=(p0, q0))
                        return mm

                    def mk_evict(e, cid, ps, q0=q0):
                        def evict():
                            st = stage_pool.tile([128, CHUNK_ROWS, WO],
                                                 dt.bfloat16)
                            nc.scalar.activation(
                                st[q0:q0 + 64, :, :],
                                ps[q0:q0 + 64, :].rearrange(
                                    "p (a b) -> p a b", a=CHUNK_ROWS),
                                mybir.ActivationFunctionType.Gelu,
                                scale=bntile[q0:q0 + 64, e, 0:1],
                                bias=bntile[q0:q0 + 64, e, 1:2])
                            nc.sync.dma_start(
                                out=out[cid, :, :, :],
                                in_=st[q0:q0 + 64, :, :])
                        return evict

                    for (e, j, cid) in sched["queue_chunks"][(h, ch)][r]:
                        ps = psum_pool.tile([128, 512], dt.float32,
                                            name="ps")
                        taps = list(_tap_offsets(e))
                        for t, (slot, ro, co) in enumerate(taps):
                            first = t == 0
                            last = t == len(taps) - 1
                            yield ("mm", mk_mm(j, ps, first, last,
                                               slot, ro, co))
                        yield ("evict", mk_evict(e, cid, ps))

            queues = [queue_events(h, ch) for h in (0, 1) for ch in (0, 1)]
            live = list(queues)
            while live:
                nxt = []
                for q in live:
                    ev = next(q, None)
                    if ev is None:
                        continue
                    ev[1]()
                    nxt.append(q)
                live = nxt

    nc.compile()
    return nc


def _host_gate(x, gate_w, gate_b):
    """Replicate reference gating in numpy (f64 pooling for robustness)."""
    pooled = x.astype(np.float64).mean(axis=(2, 3)).astype(np.float32)
    logits = pooled @ gate_w.T.astype(np.float32) + gate_b
    z = logits - logits.max(axis=1, keepdims=True)
    ez = np.exp(z.astype(np.float32))
    gates = ez / ez.sum(axis=1, keepdims=True)
    idx = np.argsort(-gates, axis=1, kind="stable")[:, :2]
    wsel = np.take_along_axis(gates, idx, axis=1)
    wsel = wsel / (wsel.sum(axis=1, keepdims=True) + 1e-8)
    return idx, wsel.astype(np.float32)


def _prep_inputs(x, ws, bs, bn_scale, bn_bias, bn_mean, bn_var, sched):
    bf16 = ml_dtypes.bfloat16
    xpad = np.zeros((B, CIN, HP, WP), dtype=bf16)
    xpad[:, :, PAD:PAD + H, PAD:PAD + W] = x.astype(bf16)

    # transposed weights, DMA-friendly layout [CIN, NTAPS, CE]
    wt = np.empty((CIN, NTAPS, CE), dtype=bf16)
    for e in range(4):
        k = KS[e]
        w = ws[e].astype(np.float32)  # [CE, CIN, k, k]
        wt[:, _SLOT_BASE[e]:_SLOT_BASE[e] + k * k, :] = (
            w.transpose(1, 2, 3, 0).reshape(CIN, k * k, CE).astype(bf16))

    # folded BN: z = conv*scale + shift
    inv = (bn_scale / np.sqrt(bn_var + BN_EPS)).astype(np.float32)
    shift = (np.stack(bs) * inv + bn_bias - bn_mean * inv).astype(np.float32)
    bnp = np.stack([inv, shift], axis=1)  # [4, 2, CE]
    bnp = np.ascontiguousarray(bnp.transpose(2, 0, 1))  # [CE, 4, 2]

    # per-core, per-half staged input rows (concatenated sample pieces)
    xps = []
    for c in range(NCORES):
        per_half = []
        for h in (0, 1):
            buf = np.zeros((CIN, max(sched["rtot"][h], 1), WP), dtype=bf16)
            for (s, halo, r_rows, roff) in sched["rounds"][h]:
                src0 = 32 * c + PAD - halo
                buf[:, roff:roff + r_rows, :] = xpad[s, :, src0:src0 + r_rows]
            per_half.append(buf)
        xps.append(per_half)
    return xps, wt, bnp


def _get_program(idx):
    key = np.asarray(idx, np.int64).tobytes()
    if key not in _CACHE:
        sched = _make_schedule(idx)
        _CACHE[key] = (sched, _build_program(sched))
    return _CACHE[key]


def run(inputs, trace=False):
    from concourse import bass_utils

    x = np.asarray(inputs["x"], dtype=np.float32)
    ws = [np.asarray(inputs[f"w{i}"], dtype=np.float32) for i in range(4)]
    bs = [np.asarray(inputs[f"b{i}"], dtype=np.float32) for i in range(4)]
    bn_scale = np.asarray(inputs["bn_scale"], dtype=np.float32)
    bn_bias = np.asarray(inputs["bn_bias"], dtype=np.float32)
    bn_mean = np.asarray(inputs["bn_mean"], dtype=np.float32)
    bn_var = np.asarray(inputs["bn_var"], dtype=np.float32)
    gate_w = np.asarray(inputs["gate_w"], dtype=np.float32)
    gate_b = np.asarray(inputs["gate_b"], dtype=np.float32)

    idx, wsel = _host_gate(x, gate_w, gate_b)
    sched, nc = _get_program(idx)
    xps, wt, bnp = _prep_inputs(x, ws, bs, bn_scale, bn_bias, bn_mean,
                                bn_var, sched)
    in_maps = []
    for c in range(NCORES):
        in_maps.append({
            "xp0": xps[c][0],
            "xp1": xps[c][1],
            "wt": wt,
            "bnp": bnp,
        })
    res = bass_utils.run_bass_kernel_spmd(
        nc, in_maps, core_ids=list(range(NCORES)), trace=trace)

    # assemble: scatter band chunks, apply top-2 gate weights, concat
    outf = np.empty((B, 2 * CE, HO, WO), dtype=np.float32)
    for c in range(NCORES):
        o = np.asarray(res.results[c]["out"], dtype=np.float32)
        for cid, (s, e, j, _ch) in enumerate(sched["chunk_map"]):
            sl = 0 if sched["idx"][s][0] == e else 1
            r0 = BAND * c + CHUNK_ROWS * j
            outf[s, sl * CE:(sl + 1) * CE, r0:r0 + CHUNK_ROWS, :] = (
                o[cid] * wsel[s, sl])
    return outf, res


def kernel(**inputs):
    outf, _ = run(inputs, trace=False)
    return outf


# revision 44
# speedup vs baseline: 1.0485x; 1.0485x over previous
"""MoE downsample kernel for 8 TRN2 NeuronCores — top-2 sparse.

The reference computes all 4 experts densely, but only the top-2 gated
experts per sample contribute to the output. Gating depends only on the
input mean-pool, so it is computed on host BEFORE compiling the device
program; the program then contains matmuls only for the selected
(sample, expert) pairs (~half the dense FLOPs for typical gatings).

Sharding: every selected (sample, expert) conv is band-sharded across
all 8 cores — core c computes output rows [16c, 16c+16). All cores
therefore execute an IDENTICAL instruction stream (SPMD-safe); only the
staged input rows differ per core. Within a core, samples are processed
in rounds streamed through SBUF: samples are exactly balanced
(subset-sum over gating costs) onto the two PE row-halves (partitions
0-63 / 64-127), and each sample's two experts are split across the two
PE col-halves so four 64x64 tile_position matmuls run concurrently
(full 128x128 array). Each strided dilated
conv is decomposed into k*k "tap" matmuls accumulated in PSUM over
512-pixel chunks; BN + conv-bias + GELU fuse into the ScalarE PSUM
eviction. Top-2 weighting and concat run on host.
"""

import numpy as np
import ml_dtypes

KS = [3, 5, 7, 9]
DS = [1, 2, 3, 4]
HALO = [d * (k - 1) // 2 for k, d in zip(KS, DS)]  # [1, 4, 9, 16]
BN_EPS = 1e-5
B, CIN, H, W = 16, 64, 256, 256
CE = 64
PAD = 16           # left/top pad (max halo); right/bottom needs 15
HP = WP = PAD + 256 + 15   # 287
HO = WO = 128
NCORES = 8
BAND = 16          # output rows per core per (sample, expert)
CHUNK_ROWS = 4     # output rows per 512-px PSUM chunk
NCHUNK = BAND // CHUNK_ROWS   # 4 chunks per (sample, expert) band
RMAX = 31 + 2 * max(HALO)     # 63 input rows per staged piece (max)
NTAPS = sum(k * k for k in KS)  # 164

# tap slot base per expert in the packed weight tensor
_SLOT_BASE = np.cumsum([0] + [k * k for k in KS]).tolist()

_CACHE = {}


def _tap_offsets(e):
    """Yield (slot, row_off, col_off) in padded coords for expert e."""
    k, d = KS[e], DS[e]
    pad = d * (k - 1) // 2
    for u in range(k):
        for v in range(k):
            slot = _SLOT_BASE[e] + u * k + v
            yield slot, d * u - pad + PAD, d * v - pad + PAD


def _make_schedule(idx):
    """Build the shared (all-core) round/queue schedule from gating."""
    idx = [(int(a), int(b)) for a, b in idx]
    costs = [KS[a] ** 2 + KS[b] ** 2 for a, b in idx]
    # exact-balance partition of samples onto the two PE row halves
    # (subset-sum DP over the 16 sample costs), fallback to LPT
    total = sum(costs)
    target = total // 2
    reach = {0: []}
    for s in range(B):
        upd = {}
        for v, mem in reach.items():
            nv = v + costs[s]
            if nv <= target and nv not in reach and nv not in upd:
                upd[nv] = mem + [s]
        reach.update(upd)
    bestv = max(reach)
    h0 = set(reach[bestv])
    halves = [sorted(h0), [s for s in range(B) if s not in h0]]
    loads = [bestv, total - bestv]
    # h0 round 0 light-ish (fast start, but window long enough that the
    # round-1 piece DMA lands in time), then heavy rounds early; h1
    # heavy-first
    halves[0].sort(key=lambda s: costs[s])
    halves[0] = halves[0][:1] + sorted(halves[0][1:], key=lambda s: -costs[s])
    halves[1].sort(key=lambda s: -costs[s])
    rounds = [[], []]                    # per half: (sample, halo, R, roff)
    rtot = [0, 0]
    for h in (0, 1):
        for s in halves[h]:
            halo = max(HALO[e] for e in idx[s])
            r_rows = 31 + 2 * halo
            rounds[h].append((s, halo, r_rows, rtot[h]))
            rtot[h] += r_rows
    # chunk ids (output slots) + per-queue per-round chunk lists
    chunk_map = []                       # cid -> (sample, expert, j, ch)
    queue_chunks = {}                    # (h, ch) -> [per-round [(e, j, cid)]]
    for h in (0, 1):
        for ch in (0, 1):
            queue_chunks[(h, ch)] = []
    for h in (0, 1):
        for (s, halo, r_rows, roff) in rounds[h]:
            ea, eb = idx[s]
            for ch in (0, 1):
                # alternate experts within each queue so ScalarE Gelu
                # evictions spread out; first chunks touch only slab 0
                lst = ([(ea, 0), (eb, 1), (ea, 2), (eb, 3)] if ch == 0
                       else [(ea, 1), (eb, 0), (ea, 3), (eb, 2)])
                entry = []
                for (e, j) in lst:
                    cid = len(chunk_map)
                    chunk_map.append((s, e, j, ch))
                    entry.append((e, j, cid))
                queue_chunks[(h, ch)].append(entry)
    return dict(idx=idx, halves=halves, loads=loads, rounds=rounds,
                rtot=rtot, chunk_map=chunk_map, queue_chunks=queue_chunks)


def _build_program(sched):
    import concourse.bass as bass  # noqa: F401
    import concourse.mybir as mybir
    import concourse.tile as tile
    from concourse import bacc
    from contextlib import ExitStack

    dt = mybir.dt
    nc = bacc.Bacc("TRN2", target_bir_lowering=False, debug=False,
                   num_devices=NCORES)
    xp = [nc.dram_tensor(f"xp{h}", [CIN, max(sched["rtot"][h], 1), WP],
                         dt.bfloat16, kind="ExternalInput") for h in (0, 1)]
    wt = nc.dram_tensor("wt", [CIN, NTAPS, CE], dt.bfloat16,
                        kind="ExternalInput")
    bnp = nc.dram_tensor("bnp", [CE, 4, 2], dt.float32, kind="ExternalInput")
    ncid = len(sched["chunk_map"])
    out = nc.dram_tensor("out", [ncid, CE, CHUNK_ROWS, WO], dt.bfloat16,
                         kind="ExternalOutput")

    with tile.TileContext(nc) as tc:
        with ExitStack() as ctx:
            consts = ctx.enter_context(tc.tile_pool(name="consts", bufs=1))
            px0 = ctx.enter_context(tc.tile_pool(name="px0", bufs=2))
            px1 = ctx.enter_context(tc.tile_pool(name="px1", bufs=2))
            piece_pools = [px0, px1]
            stage_pool = ctx.enter_context(tc.tile_pool(name="st", bufs=8))

            wtile = consts.tile([128, NTAPS, CE], dt.bfloat16)
            bntile = consts.tile([128, 4, 2], dt.float32)

            psum_pool = ctx.enter_context(
                tc.tile_pool(name="ps", bufs=8, space="PSUM"))

            piece_shared = [{}, {}]   # h -> round -> sbuf tile

            def stage_piece(h, r, lo, hi, alloc=False, eng=None):
                """DMA rows [lo,hi) of the round-r piece for half h."""
                s, halo, r_rows, roff = sched["rounds"][h][r]
                p0 = h * 64
                hi = min(hi, r_rows)
                if alloc:
                    pt = piece_pools[h].tile([128, RMAX, WP], dt.bfloat16)
                    piece_shared[h][r] = pt
                else:
                    pt = piece_shared[h][r]
                if lo >= hi:
                    return
                (eng or nc.gpsimd).dma_start(
                    out=pt[p0:p0 + 64, lo:hi, :],
                    in_=xp[h][:, roff + lo:roff + hi, :])

            def stage_weights(h, e, t0=0, t1=None, eng=None):
                p0 = h * 64
                sb = _SLOT_BASE[e]
                ke = KS[e] * KS[e]
                t1 = ke if t1 is None else min(t1, ke)
                if t0 >= t1:
                    return
                (eng or nc.gpsimd).dma_start(
                    out=wtile[p0:p0 + 64, sb + t0:sb + t1, :],
                    in_=wt[:, sb + t0:sb + t1, :])

            # ---- prologue: order DMAs so the first matmuls start early ----
            first_use = [[], []]       # per half: experts by first use
            for h in (0, 1):
                for (s, _h_, _r_, _o_) in sched["rounds"][h]:
                    for e in sched["idx"][s]:
                        if e not in first_use[h]:
                            first_use[h].append(e)
            hw = nc.gpsimd
            # critical path: h0 round-0 first-chunk deps, then h1's
            for h in (0, 1):
                if not sched["rounds"][h]:
                    continue
                halo0 = sched["rounds"][h][0][1]
                stage_weights(h, first_use[h][0], 0, 16, eng=hw)
                stage_piece(h, 0, 0, 7 + 2 * halo0, alloc=True, eng=hw)
            # bn params gate every eviction (needed ~6us after first MM)
            for h in (0, 1):
                hw.dma_start(out=bntile[h * 64:h * 64 + 64, :, :],
                             in_=bnp.ap())
            # warm the PE clock (HAM) with dummy matmuls while DMA streams;
            # the psum slot recycles via the pool ring, it is never read
            if first_use[0]:
                wsb = _SLOT_BASE[first_use[0][0]]
                ps = psum_pool.tile([128, 512], dt.float32)
                for _ in range(14):
                    nc.tensor.matmul(ps[0:64, 0:64],
                                     wtile[0:64, wsb, :],
                                     wtile[0:64, wsb, :],
                                     start=True, stop=True,
                                     tile_position=(0, 0))
            for h in (0, 1):
                if not sched["rounds"][h]:
                    continue
                halo0 = sched["rounds"][h][0][1]
                stage_weights(h, first_use[h][0], 16, None, eng=hw)
                stage_piece(h, 0, 7 + 2 * halo0, 15 + 2 * halo0, eng=hw)
            for h in (0, 1):           # second expert + rest of round 0
                if len(first_use[h]) > 1:
                    stage_weights(h, first_use[h][1], eng=hw)
                if sched["rounds"][h]:
                    halo0 = sched["rounds"][h][0][1]
                    stage_piece(h, 0, 15 + 2 * halo0, RMAX, eng=hw)
            # experts first needed in round >= 1 are staged from the pump,
            # one round ahead — keeps the prologue DMA backlog small
            first_round = [{}, {}]
            for h in (0, 1):
                for r, (s, _h_, _r_, _o_) in enumerate(sched["rounds"][h]):
                    for e in sched["idx"][s]:
                        first_round[h].setdefault(e, r)

            def queue_events(h, ch):
                p0 = h * 64            # rhs/lhsT partitions (PE rows)
                q0 = ch * 64           # psum/out partitions (PE cols)
                for r, (s, halo, r_rows, roff) in enumerate(
                        sched["rounds"][h]):
                    if ch == 0 and r + 1 < len(sched["rounds"][h]):
                        # prefetch next round's piece + its new experts'
                        # weights, critical-first: first taps of the first
                        # expert and the first row slab ahead of the bulk
                        h1rows = 15 + 2 * sched["rounds"][h][r + 1][1]
                        wes = [e for e, fr in first_round[h].items()
                               if fr == r + 1]
                        a_next = sched["idx"][sched["rounds"][h][r + 1][0]][0]

                        def stage(h=h, r=r, h1rows=h1rows, wes=wes,
                                  a_next=a_next):
                            if a_next in wes:
                                stage_weights(h, a_next, 0, 16)
                            stage_piece(h, r + 1, 0, h1rows, alloc=True)
                            if a_next in wes:
                                stage_weights(h, a_next, 16, None)
                            for e in wes:
                                if e != a_next:
                                    stage_weights(h, e)
                            stage_piece(h, r + 1, h1rows, RMAX)
                        yield ("dma", stage)
                    else:
                        yield ("noop", lambda: None)  # keep queues in lockstep
                    def mk_mm(j, ps, first, last, slot, ro, co,
                              h=h, r=r, halo=halo, p0=p0, q0=q0):
                        rl = 8 * j + ro - PAD + halo

                        def mm():
                            pt = piece_shared[h][r]
                            rhs = pt[p0:p0 + 64,
                                     rl:rl + 2 * CHUNK_ROWS - 1:2,
                                     co:co + 2 * WO - 1:2]
                            lhsT = wtile[p0:p0 + 64, slot, :]
                            nc.tensor.matmul(ps[q0:q0 + 64, :], lhsT, rhs,
                                             start=first, stop=last,
                                             tile_position=(p0, q0))
                        return mm

                    def mk_evict(e, cid, ps, q0=q0):
                        def evict():
                            st = stage_pool.tile([128, CHUNK_ROWS, WO],
                                                 dt.bfloat16)
                            nc.scalar.activation(
                                st[q0:q0 + 64, :, :],
                                ps[q0:q0 + 64, :].rearrange(
                                    "p (a b) -> p a b", a=CHUNK_ROWS),
                                mybir.ActivationFunctionType.Gelu,
                                scale=bntile[q0:q0 + 64, e, 0:1],
                                bias=bntile[q0:q0 + 64, e, 1:2])
                            nc.sync.dma_start(
                                out=out[cid, :, :, :],
                                in_=st[q0:q0 + 64, :, :])
                        return evict

                    for (e, j, cid) in sched["queue_chunks"][(h, ch)][r]:
                        ps = psum_pool.tile([128, 512], dt.float32,
                                            name="ps")
                        taps = list(_tap_offsets(e))
                        for t, (slot, ro, co) in enumerate(taps):
                            first = t == 0
                            last = t == len(taps) - 1
                            yield ("mm", mk_mm(j, ps, first, last,
                                               slot, ro, co))
                        yield ("evict", mk_evict(e, cid, ps))

            queues = [queue_events(h, ch) for h in (0, 1) for ch in (0, 1)]
            live = list(queues)
            while live:
                nxt = []
                for q in live:
                    ev = next(q, None)
                    if ev is None:
                        continue
                    ev[1]()
                    nxt.append(q)
                live = nxt

    nc.compile()
    return nc


def _host_gate(x, gate_w, gate_b):
    """Replicate reference gating in numpy (f64 pooling for robustness)."""
    pooled = x.astype(np.float64).mean(axis=(2, 3)).astype(np.float32)
    logits = pooled @ gate_w.T.astype(np.float32) + gate_b
    z = logits - logits.max(axis=1, keepdims=True)
    ez = np.exp(z.astype(np.float32))
    gates = ez / ez.sum(axis=1, keepdims=True)
    idx = np.argsort(-gates, axis=1, kind="stable")[:, :2]
    wsel = np.take_along_axis(gates, idx, axis=1)
    wsel = wsel / (wsel.sum(axis=1, keepdims=True) + 1e-8)
    return idx, wsel.astype(np.float32)


def _prep_inputs(x, ws, bs, bn_scale, bn_bias, bn_mean, bn_var, sched):
    bf16 = ml_dtypes.bfloat16
    xpad = np.zeros((B, CIN, HP, WP), dtype=bf16)
    xpad[:, :, PAD:PAD + H, PAD:PAD + W] = x.astype(bf16)

    # transposed weights, DMA-friendly layout [CIN, NTAPS, CE]
    wt = np.empty((CIN, NTAPS, CE), dtype=bf16)
    for e in range(4):
        k = KS[e]
        w = ws[e].astype(np.float32)  # [CE, CIN, k, k]
        wt[:, _SLOT_BASE[e]:_SLOT_BASE[e] + k * k, :] = (
            w.transpose(1, 2, 3, 0).reshape(CIN, k * k, CE).astype(bf16))

    # folded BN: z = conv*scale + shift
    inv = (bn_scale / np.sqrt(bn_var + BN_EPS)).astype(np.float32)
    shift = (np.stack(bs) * inv + bn_bias - bn_mean * inv).astype(np.float32)
    bnp = np.stack([inv, shift], axis=1)  # [4, 2, CE]
    bnp = np.ascontiguousarray(bnp.transpose(2, 0, 1))  # [CE, 4, 2]

    # per-core, per-half staged input rows (concatenated sample pieces)
    xps = []
    for c in range(NCORES):
        per_half = []
        for h in (0, 1):
            buf = np.zeros((CIN, max(sched["rtot"][h], 1), WP), dtype=bf16)
            for (s, halo, r_rows, roff) in sched["rounds"][h]:
                src0 = 32 * c + PAD - halo
                buf[:, roff:roff + r_rows, :] = xpad[s, :, src0:src0 + r_rows]
            per_half.append(buf)
        xps.append(per_half)
    return xps, wt, bnp


def _get_program(idx):
    key = np.asarray(idx, np.int64).tobytes()
    if key not in _CACHE:
        sched = _make_schedule(idx)
        _CACHE[key] = (sched, _build_program(sched))
    return _CACHE[key]


def run(inputs, trace=False):
    from concourse import bass_utils

    x = np.asarray(inputs["x"], dtype=np.float32)
    ws = [np.asarray(inputs[f"w{i}"], dtype=np.float32) for i in range(4)]
    bs = [np.asarray(inputs[f"b{i}"], dtype=np.float32) for i in range(4)]
    bn_scale = np.asarray(inputs["bn_scale"], dtype=np.float32)
    bn_bias = np.asarray(inputs["bn_bias"], dtype=np.float32)
    bn_mean = np.asarray(inputs["bn_mean"], dtype=np.float32)
    bn_var = np.asarray(inputs["bn_var"], dtype=np.float32)
    gate_w = np.asarray(inputs["gate_w"], dtype=np.float32)
    gate_b = np.asarray(inputs["gate_b"], dtype=np.float32)

    idx, wsel = _host_gate(x, gate_w, gate_b)
    sched, nc = _get_program(idx)
    xps, wt, bnp = _prep_inputs(x, ws, bs, bn_scale, bn_bias, bn_mean,
                                bn_var, sched)
    in_maps = []
    for c in range(NCORES):
        in_maps.append({
            "xp0": xps[c][0],
            "xp1": xps[c][1],
            "wt": wt,
            "bnp": bnp,
        })
    res = bass_utils.run_bass_kernel_spmd(
        nc, in_maps, core_ids=list(range(NCORES)), trace=trace)

    # assemble: scatter band chunks, apply top-2 gate weights, concat
    outf = np.empty((B, 2 * CE, HO, WO), dtype=np.float32)
    for c in range(NCORES):
        o = np.asarray(res.results[c]["out"], dtype=np.float32)
        for cid, (s, e, j, _ch) in enumerate(sched["chunk_map"]):
            sl = 0 if sched["idx"][s][0] == e else 1
            r0 = BAND * c + CHUNK_ROWS * j
            outf[s, sl * CE:(sl + 1) * CE, r0:r0 + CHUNK_ROWS, :] = (
                o[cid] * wsel[s, sl])
    return outf, res


def kernel(**inputs):
    outf, _ = run(inputs, trace=False)
    return outf
